# revision 1
# baseline (speedup 1.0000x reference)
"""BatchHardTripletLoss on 8 TRN2 NeuronCores (raw Bass, explicit sync).

N=8192 anchors, D=128, 512 labels. Each core owns 1024 anchor rows and
computes its [1024, 8192] block of t = (sq_j - 2<e_r,e_j>) + BIG*same,
then per-row max/min. Mining on squared distances is order-equivalent to
mining on distances (sqrt is monotone), and the per-row constant sq_r
shifts every column equally so it is re-added after the reduction:
  hardest_pos_d2 = max_j(t) - BIG + sq_r
  hardest_neg_d2 = min_j(t) + sq_r
The block lands in PSUM via two accumulating matmuls (gram with
-2-scaled rows, plus a K=1 rank-one update carrying sq_j); the vector
engine adds BIG*(label_col==label_row) and reduces max/min per anchor.
Host does the final sqrt / relu / valid-mean over 8192 values.

Raw Bass with standalone wait_ge instructions: the walrus backend only
encodes one embedded semaphore wait per compute instruction, which the
Tile scheduler's output exceeds.  Every DVE instruction increments
sem_v, every PE matmul group increments sem_p; waits are computed from a
static tick schedule.
"""

import numpy as np

N = 8192
D = 128
NCORES = 8
ROWS = N // NCORES          # 1024 rows per core
RCHUNKS = ROWS // 128       # 8 row chunks of 128
CCHUNK = 512                # psum bank = 512 f32 columns
NCCHUNKS = N // CCHUNK      # 16 col chunks
NBANKS = 4
BIG = 4096.0
MARGIN = 0.3

_cache = {}


def _build():
    import contextlib
    import concourse.bass as bass
    from concourse import mybir

    fp32 = mybir.dt.float32
    Alu = mybir.AluOpType
    AX = mybir.AxisListType.X

    nc = bass.Bass()

    embT_in = nc.dram_tensor("embT_in", [D, N], fp32, kind="ExternalInput")
    rowsT_in = nc.dram_tensor("rowsT_in", [D, ROWS], fp32,
                              kind="ExternalInput")
    labels_all = nc.dram_tensor("labels_all", [N], fp32, kind="ExternalInput")
    rows_labels = nc.dram_tensor("rows_labels", [ROWS], fp32,
                                 kind="ExternalInput")
    out = nc.dram_tensor("out", [128, 2 * RCHUNKS], fp32,
                         kind="ExternalOutput")
    out_sqr = nc.dram_tensor("out_sqr", [1, ROWS], fp32, kind="ExternalOutput")

    # --- static tick schedule -------------------------------------------
    # DVE (sem_v): 2 memsets; per k in 16: square_k, copy_k; rowsT2; rT2sq;
    # 2 sqr copies; then per idx: TS, TT, TRmax, TRmin; +2 finals per i.
    V_SETUP = 2 + 32 + 2 + 2                      # = 38
    NIDX = RCHUNKS * NCCHUNKS                     # 128

    def tt_tick(idx):
        i, j = divmod(idx, NCCHUNKS)
        return V_SETUP + (4 * NCCHUNKS + 2) * i + 4 * j + 2

    V_FINAL = V_SETUP + (4 * NCCHUNKS + 2) * RCHUNKS
    # PE (sem_p): 16 sqc matmuls, 2 sqr matmuls, then 1 per idx (mm2).
    P_SETUP = NCCHUNKS + 2                        # = 18

    ctx = contextlib.ExitStack()
    with ctx:
        sb = lambda nm, shape: ctx.enter_context(
            nc.sbuf_tensor(nm, shape, fp32))
        ps = lambda nm, shape: ctx.enter_context(
            nc.psum_tensor(nm, shape, fp32))
        sem = lambda nm: ctx.enter_context(nc.semaphore(name=nm))

        embT = sb("embT", [128, N])
        lab_bc = sb("lab_bc", [128, N])
        sqc_row = sb("sqc_row", [1, N])
        rowsT_sb = sb("rowsT_sb", [128, ROWS])
        rowsT2 = sb("rowsT2", [128, ROWS])
        rT2sq = sb("rT2sq", [128, ROWS])
        sqr_row = sb("sqr_row", [1, ROWS])
        rowlab = sb("rowlab", [128, RCHUNKS])
        ones01 = sb("ones01", [128, 1])
        ones128 = sb("ones128", [1, 128])
        sqsc = [sb(f"sqsc{w}", [128, CCHUNK]) for w in range(2)]
        sbm = [sb(f"sbm{w}", [128, CCHUNK]) for w in range(2)]
        tm = [sb(f"tm{w}", [128, CCHUNK]) for w in range(2)]
        maxpart = sb("maxpart", [128, NCCHUNKS])
        minpart = sb("minpart", [128, NCCHUNKS])
        outsb = sb("outsb", [128, 2 * RCHUNKS])

        psum = [ps(f"psum{b}", [128, CCHUNK]) for b in range(NBANKS)]

        s_emb = sem("s_emb")
        s_rows = sem("s_rows")
        s_lab = sem("s_lab")
        s_rlab = sem("s_rlab")
        s_out = sem("s_out")
        sem_v = sem("sem_v")
        sem_p = sem("sem_p")

        import concourse.bass as bass_mod
        lab_ap = labels_all[:]
        lab_bcast_src = bass_mod.AP(
            tensor=lab_ap.tensor, offset=lab_ap.offset,
            ap=[[0, 128]] + list(lab_ap.ap),
        )

        with nc.Block() as block:

            @block.sync
            def _(sync):
                sync.dma_start(out=embT[:, :], in_=embT_in[:, :]).then_inc(
                    s_emb, 16)
                sync.wait_ge(sem_v, V_SETUP)
                sync.dma_start(out=out_sqr[:, :], in_=sqr_row[:, :]).then_inc(
                    s_out, 16)
                sync.wait_ge(sem_v, V_FINAL)
                sync.dma_start(out=out[:, :], in_=outsb[:, :]).then_inc(
                    s_out, 16)
                sync.wait_ge(s_out, 32)

            @block.scalar
            def _(scalar):
                scalar.dma_start(out=lab_bc[:, :], in_=lab_bcast_src).then_inc(
                    s_lab, 16)

            @block.gpsimd
            def _(gpsimd):
                gpsimd.dma_start(out=rowsT_sb[:, :],
                                 in_=rowsT_in[:, :]).then_inc(s_rows, 16)
                with nc.allow_non_contiguous_dma(reason="1KB rowlab gather"):
                    gpsimd.dma_start(
                        out=rowlab[:, :],
                        in_=rows_labels[:].rearrange("(i p) -> p i", p=128),
                    ).then_inc(s_rlab, 16)

            @block.tensor
            def _(tensor):
                for k in range(NCCHUNKS):
                    tensor.wait_ge(sem_v, 3 + 2 * k)
                    tensor.matmul(psum[k % 2][0:1, :], ones01[:, :],
                                  sqsc[k % 2][:, :], start=True,
                                  stop=True).then_inc(sem_p)
                for k in range(2):
                    tensor.wait_ge(sem_v, 36)
                    tensor.matmul(psum[k][0:1, :], ones01[:, :],
                                  rT2sq[:, k * CCHUNK:(k + 1) * CCHUNK],
                                  start=True, stop=True).then_inc(sem_p)
                for idx in range(NIDX):
                    i, j = divmod(idx, NCCHUNKS)
                    b = idx % NBANKS
                    js = slice(j * CCHUNK, (j + 1) * CCHUNK)
                    if idx < NBANKS:
                        tensor.wait_ge(sem_v, V_SETUP)
                    else:
                        tensor.wait_ge(sem_v, tt_tick(idx - NBANKS))
                    tensor.matmul(psum[b][:, :],
                                  rowsT2[:, i * 128:(i + 1) * 128],
                                  embT[:, js], start=True, stop=False)
                    tensor.matmul(psum[b][:, :], ones128[:, :],
                                  sqc_row[:, js], start=False,
                                  stop=True).then_inc(sem_p)

            @block.vector
            def _(vector):
                # Every DVE op waits for the previous one (sem_v self-chain):
                # the DVE pipe drains between ops anyway, and both the race
                # detector and HW want the sync explicit.
                v = 0

                def chain(ins):
                    nonlocal v
                    ins.then_inc(sem_v)
                    v += 1
                    vector.wait_ge(sem_v, v)

                chain(vector.memset(ones01[:, :], 1.0))             # v1
                chain(vector.memset(ones128[:, :], 1.0))            # v2
                vector.wait_ge(s_emb, 16)
                for k in range(NCCHUNKS):
                    ks = slice(k * CCHUNK, (k + 1) * CCHUNK)
                    if k >= 2:
                        # WAR: matmul k-2 must be done reading sqsc[k%2]
                        vector.wait_ge(sem_p, k - 1)
                    chain(vector.tensor_mul(sqsc[k % 2][:, :], embT[:, ks],
                                            embT[:, ks]))           # 3+2k
                    vector.wait_ge(sem_p, k + 1)
                    chain(vector.tensor_copy(sqc_row[0:1, ks],
                                             psum[k % 2][0:1, :]))  # 4+2k
                vector.wait_ge(s_rows, 16)
                chain(vector.tensor_scalar_mul(rowsT2[:, :], rowsT_sb[:, :],
                                               -2.0))               # 35
                chain(vector.tensor_mul(rT2sq[:, :], rowsT_sb[:, :],
                                        rowsT_sb[:, :]))            # 36
                for k in range(2):
                    ks = slice(k * CCHUNK, (k + 1) * CCHUNK)
                    vector.wait_ge(sem_p, P_SETUP - 1 + k)
                    chain(vector.tensor_copy(sqr_row[0:1, ks],
                                             psum[k][0:1, :]))      # 37, 38
                vector.wait_ge(s_lab, 16)
                vector.wait_ge(s_rlab, 16)
                for idx in range(NIDX):
                    i, j = divmod(idx, NCCHUNKS)
                    b = idx % NBANKS
                    w = idx % 2
                    js = slice(j * CCHUNK, (j + 1) * CCHUNK)
                    chain(vector.tensor_scalar(
                        out=sbm[w][:, :], in0=lab_bc[:, js],
                        scalar1=rowlab[:, i:i + 1], scalar2=BIG,
                        op0=Alu.is_equal, op1=Alu.mult))
                    vector.wait_ge(sem_p, P_SETUP + idx + 1)
                    chain(vector.tensor_add(tm[w][:, :], sbm[w][:, :],
                                            psum[b][:, :]))
                    chain(vector.tensor_reduce(out=maxpart[:, j:j + 1],
                                               in_=tm[w][:, :], axis=AX,
                                               op=Alu.max))
                    chain(vector.tensor_reduce(out=minpart[:, j:j + 1],
                                               in_=tm[w][:, :], axis=AX,
                                               op=Alu.min))
                    if j == NCCHUNKS - 1:
                        chain(vector.tensor_reduce(out=outsb[:, i:i + 1],
                                                   in_=maxpart[:, :], axis=AX,
                                                   op=Alu.max))
                        chain(vector.tensor_reduce(
                            out=outsb[:, RCHUNKS + i:RCHUNKS + i + 1],
                            in_=minpart[:, :], axis=AX,
                            op=Alu.min))

    return nc


def _get_nc():
    if "nc" not in _cache:
        _cache["nc"] = _build()
    return _cache["nc"]


def _postprocess(outs, sqrs, labels_i64):
    # outs: list of NCORES arrays [128, 2*RCHUNKS]; sqrs: [1, ROWS] each
    tmax = np.empty(N, np.float32)
    tmin = np.empty(N, np.float32)
    sqr = np.empty(N, np.float32)
    for c in range(NCORES):
        o = outs[c]
        sqr[c * ROWS:(c + 1) * ROWS] = sqrs[c][0]
        for i in range(RCHUNKS):
            r0 = c * ROWS + i * 128
            tmax[r0:r0 + 128] = o[:, i]
            tmin[r0:r0 + 128] = o[:, RCHUNKS + i]
    hp_d2 = tmax - np.float32(BIG) + sqr
    hn_d2 = tmin + sqr
    hp = np.sqrt(np.maximum(hp_d2, 0.0), dtype=np.float32)
    hn = np.sqrt(np.maximum(hn_d2, 0.0), dtype=np.float32)
    loss = np.maximum(hp - hn + np.float32(MARGIN), 0.0).astype(np.float32)

    labels_i64 = labels_i64.astype(np.int64)
    counts = np.bincount(labels_i64, minlength=1)
    csame = counts[labels_i64]
    valid = (csame > 1) & (csame < N)
    cnt = np.float32(valid.sum())
    if cnt > 0:
        return np.array(loss[valid].sum() / max(cnt, np.float32(1.0)),
                        np.float32)
    return np.array(loss.mean(), np.float32)


def _make_in_maps(embeddings, labels_f32):
    embT = np.ascontiguousarray(embeddings.T)
    in_maps = []
    for c in range(NCORES):
        in_maps.append({
            "embT_in": embT,
            "labels_all": labels_f32,
            "rowsT_in": np.ascontiguousarray(embT[:, c * ROWS:(c + 1) * ROWS]),
            "rows_labels": np.ascontiguousarray(
                labels_f32[c * ROWS:(c + 1) * ROWS]),
        })
    return in_maps


def kernel(embeddings, labels):
    from concourse.bass_utils import run_bass_kernel_spmd

    embeddings = np.asarray(embeddings, np.float32)
    labels = np.asarray(labels)
    labels_f32 = labels.astype(np.float32)

    nc = _get_nc()
    res = run_bass_kernel_spmd(nc, _make_in_maps(embeddings, labels_f32),
                               list(range(NCORES)))
    outs = [np.asarray(res.results[c]["out"]) for c in range(NCORES)]
    sqrs = [np.asarray(res.results[c]["out_sqr"]) for c in range(NCORES)]
    return _postprocess(outs, sqrs, labels)



# revision 2
# speedup vs baseline: 1.9346x; 1.9346x over previous
"""BatchHardTripletLoss on 8 TRN2 NeuronCores — v2 (clean DMAs, host
pre/post, minimal device program).

N=8192 anchors, D=128, 512 labels. Each core owns 1024 anchor rows and
computes its [1024, 8192] block of t = (sq_j - 2<e_r,e_j>) + BIG*same,
then per-row max/min. Mining on squared distances is order-equivalent
(sqrt monotone); the per-row constant sq_r shifts every column equally
and is re-added on the host:
  hardest_pos_d2 = max_j(t) - BIG + sq_r
  hardest_neg_d2 = min_j(t) + sq_r

Differences vs v1: all DMAs are contiguous (host materializes the label
broadcast, the per-chunk row-label layout, -2x row slices, and the
column squared-norms), the on-device setup phase is gone, and DVE ops
rely on same-queue ordering instead of a per-op semaphore self-chain.
Host does the final sqrt / relu / valid-mean over 8192 values.
"""

import numpy as np

N = 8192
D = 128
NCORES = 8
ROWS = N // NCORES          # 1024 rows per core
RCHUNKS = ROWS // 128       # 8 row chunks of 128
CCHUNK = 512                # psum bank = 512 f32 columns
NCCHUNKS = N // CCHUNK      # 16 col chunks
NBANKS = 4
NIDX = RCHUNKS * NCCHUNKS   # 128
BIG = 4096.0
MARGIN = 0.3

_cache = {}


def _build():
    import contextlib
    import concourse.bass as bass
    from concourse import mybir

    fp32 = mybir.dt.float32
    Alu = mybir.AluOpType
    AX = mybir.AxisListType.X

    nc = bass.Bass()
    embT_in = nc.dram_tensor("embT_in", [D, N], fp32, kind="ExternalInput")
    rowsT2_in = nc.dram_tensor("rowsT2_in", [D, ROWS], fp32,
                               kind="ExternalInput")
    sq_in = nc.dram_tensor("sq_in", [1, N], fp32, kind="ExternalInput")
    lab_bc_in = nc.dram_tensor("lab_bc_in", [128, N], fp32,
                               kind="ExternalInput")
    rowlab_in = nc.dram_tensor("rowlab_in", [128, RCHUNKS], fp32,
                               kind="ExternalInput")
    out = nc.dram_tensor("out", [128, 2 * RCHUNKS], fp32,
                         kind="ExternalOutput")

    # Every DVE op self-chains on sem_v (the race detector wants explicit
    # sync; HW cost is nil — same-queue ops are ordered anyway).
    # Ops: memset(1); per idx k=16i+j: mask, add, rmax, rmin (4);
    # plus 2 finals after each completed row chunk.
    # After add(k): sem_v = 1 + 4k + 2*(k//16) + 2.
    def v_after_add(k):
        return 4 * k + 2 * (k // NCCHUNKS) + 3

    V_FINAL = 1 + 4 * NIDX + 2 * RCHUNKS      # 529

    ctx = contextlib.ExitStack()
    with ctx:
        sb = lambda nm, shape: ctx.enter_context(
            nc.sbuf_tensor(nm, shape, fp32))
        ps = lambda nm, shape: ctx.enter_context(
            nc.psum_tensor(nm, shape, fp32))
        sem = lambda nm: ctx.enter_context(nc.semaphore(name=nm))

        embT = sb("embT", [128, N])
        rowsT2 = sb("rowsT2", [128, ROWS])
        sq_row = sb("sq_row", [1, N])
        lab_bc = sb("lab_bc", [128, N])
        rowlab = sb("rowlab", [128, RCHUNKS])
        ones128 = sb("ones128", [1, 128])
        sbm = [sb(f"sbm{w}", [128, CCHUNK]) for w in range(2)]
        tm = [sb(f"tm{w}", [128, CCHUNK]) for w in range(2)]
        maxpart = sb("maxpart", [128, NCCHUNKS])
        minpart = sb("minpart", [128, NCCHUNKS])
        outsb = sb("outsb", [128, 2 * RCHUNKS])
        psum = [ps(f"psum{b}", [128, CCHUNK]) for b in range(NBANKS)]

        s_d = sem("s_d")
        sem_v = sem("sem_v")
        sem_p = sem("sem_p")
        s_out = sem("s_out")

        with nc.Block() as block:
            @block.sync
            def _(sync):
                sync.dma_start(out=embT[:, :], in_=embT_in[:, :]).then_inc(
                    s_d, 16)
                sync.dma_start(out=rowsT2[:, :],
                               in_=rowsT2_in[:, :]).then_inc(s_d, 16)
                sync.dma_start(out=sq_row[:, :], in_=sq_in[:, :]).then_inc(
                    s_d, 16)
                sync.dma_start(out=lab_bc[:, :],
                               in_=lab_bc_in[:, :]).then_inc(s_d, 16)
                sync.dma_start(out=rowlab[:, :],
                               in_=rowlab_in[:, :]).then_inc(s_d, 16)
                sync.wait_ge(sem_v, V_FINAL)
                sync.dma_start(out=out[:, :], in_=outsb[:, :]).then_inc(
                    s_out, 16)
                sync.wait_ge(s_out, 16)

            @block.tensor
            def _(tensor):
                tensor.wait_ge(s_d, 80)
                tensor.wait_ge(sem_v, 1)
                for idx in range(NIDX):
                    i, j = divmod(idx, NCCHUNKS)
                    b = idx % NBANKS
                    js = slice(j * CCHUNK, (j + 1) * CCHUNK)
                    if idx >= NBANKS:
                        # WAR: DVE's add for idx-NBANKS must have read bank b
                        tensor.wait_ge(sem_v, v_after_add(idx - NBANKS))
                    tensor.matmul(psum[b][:, :],
                                  rowsT2[:, i * 128:(i + 1) * 128],
                                  embT[:, js], start=True, stop=False)
                    tensor.matmul(psum[b][:, :], ones128[:, :],
                                  sq_row[:, js], start=False,
                                  stop=True).then_inc(sem_p)

            @block.vector
            def _(vector):
                v = 0

                def chain(ins):
                    nonlocal v
                    ins.then_inc(sem_v)
                    v += 1
                    vector.wait_ge(sem_v, v)

                vector.wait_ge(s_d, 80)
                chain(vector.memset(ones128[:, :], 1.0))
                for idx in range(NIDX):
                    i, j = divmod(idx, NCCHUNKS)
                    b = idx % NBANKS
                    w = idx % 2
                    js = slice(j * CCHUNK, (j + 1) * CCHUNK)
                    chain(vector.tensor_scalar(
                        out=sbm[w][:, :], in0=lab_bc[:, js],
                        scalar1=rowlab[:, i:i + 1], scalar2=BIG,
                        op0=Alu.is_equal, op1=Alu.mult))
                    vector.wait_ge(sem_p, idx + 1)
                    chain(vector.tensor_add(tm[w][:, :], sbm[w][:, :],
                                            psum[b][:, :]))
                    assert v == v_after_add(idx)
                    chain(vector.tensor_reduce(out=maxpart[:, j:j + 1],
                                               in_=tm[w][:, :], axis=AX,
                                               op=Alu.max))
                    chain(vector.tensor_reduce(out=minpart[:, j:j + 1],
                                               in_=tm[w][:, :], axis=AX,
                                               op=Alu.min))
                    if j == NCCHUNKS - 1:
                        chain(vector.tensor_reduce(
                            out=outsb[:, i:i + 1], in_=maxpart[:, :],
                            axis=AX, op=Alu.max))
                        chain(vector.tensor_reduce(
                            out=outsb[:, RCHUNKS + i:RCHUNKS + i + 1],
                            in_=minpart[:, :], axis=AX,
                            op=Alu.min))
                assert v == V_FINAL

    return nc


def _get_nc():
    if "nc" not in _cache:
        _cache["nc"] = _build()
    return _cache["nc"]


def _postprocess(outs, sq, labels_i64):
    tmax = np.empty(N, np.float32)
    tmin = np.empty(N, np.float32)
    for c in range(NCORES):
        o = outs[c]
        for i in range(RCHUNKS):
            r0 = c * ROWS + i * 128
            tmax[r0:r0 + 128] = o[:, i]
            tmin[r0:r0 + 128] = o[:, RCHUNKS + i]
    hp_d2 = tmax - np.float32(BIG) + sq
    hn_d2 = tmin + sq
    hp = np.sqrt(np.maximum(hp_d2, 0.0), dtype=np.float32)
    hn = np.sqrt(np.maximum(hn_d2, 0.0), dtype=np.float32)
    loss = np.maximum(hp - hn + np.float32(MARGIN), 0.0).astype(np.float32)

    labels_i64 = labels_i64.astype(np.int64)
    counts = np.bincount(labels_i64, minlength=1)
    csame = counts[labels_i64]
    valid = (csame > 1) & (csame < N)
    cnt = np.float32(valid.sum())
    if cnt > 0:
        return np.array(loss[valid].sum() / max(cnt, np.float32(1.0)),
                        np.float32)
    return np.array(loss.mean(), np.float32)


def _make_in_maps(embeddings, labels_f32):
    embT = np.ascontiguousarray(embeddings.T)
    sq = np.sum(embeddings * embeddings, axis=1).astype(np.float32)
    lab_bc = np.ascontiguousarray(
        np.broadcast_to(labels_f32[None, :], (128, N)))
    in_maps = []
    for c in range(NCORES):
        rs = slice(c * ROWS, (c + 1) * ROWS)
        in_maps.append({
            "embT_in": embT,
            "rowsT2_in": np.ascontiguousarray(-2.0 * embT[:, rs]),
            "sq_in": sq[None, :],
            "lab_bc_in": lab_bc,
            "rowlab_in": np.ascontiguousarray(
                labels_f32[rs].reshape(RCHUNKS, 128).T),
        })
    return in_maps


def kernel(embeddings, labels):
    from concourse.bass_utils import run_bass_kernel_spmd

    embeddings = np.asarray(embeddings, np.float32)
    labels = np.asarray(labels)
    labels_f32 = labels.astype(np.float32)

    nc = _get_nc()
    in_maps = _make_in_maps(embeddings, labels_f32)
    sq = np.sum(embeddings * embeddings, axis=1).astype(np.float32)
    res = run_bass_kernel_spmd(nc, in_maps, list(range(NCORES)))
    outs = [np.asarray(res.results[c]["out"]) for c in range(NCORES)]
    return _postprocess(outs, sq, labels)
